# revision 42
# baseline (speedup 1.0000x reference)
"""Trainium2 Bass kernel for nn_LCN (locally-connected network).

Computation (see module docstring math):
  x: (512, 1, 280, 280) -> non-overlapping 28x28 patches (10x10 grid, P=100)
  y[b, f, p] = sum_{k,l} x[b, 28ph+k, 28pw+l] * w[f*100+p, 0, k, l]
  y = relu(y + bias[f*100+p]);  out = y_flat @ dec_w.T + dec_b   (j = f*100 + p)

Sharding: 8 cores = 4 batch groups x 2 image halves (rows 0..139 | 140..279).
Each core: 128 images, 5 bands (28 rows each), 50 patches.
Per core pipeline:
  - DMA band [128b, 7840] (fp32, contiguous in HBM)
  - PE transpose x chunks [128b, 112pix] -> PSUM [112, 128] (identity matmul)
  - DVE/ACT evacuate PSUM -> SBUF
  - per patch: 7 accumulating matmuls lhsT=w[112,16], rhs=xT[112,128] -> y PSUM
    (4 patches per PSUM tile at partition offsets 0/32/64/96)
  - ACT: relu(y + bias) -> y_sb  [j on partitions, gappy layout]
  - decoder: 13 accumulating matmuls lhsT=dec[128,10] (zeros in gaps) -> out [10,128]
Host sums the two half-image partial decoder outputs and adds dec_b.
"""

import sys

import numpy as np

for _p in ("/opt/trn_rl_repo", "/opt/trn_rl_repo/concourse"):
    if _p not in sys.path:
        sys.path.insert(0, _p)

import concourse.bass as bass
import concourse.mybir as mybir
import concourse.tile as tile
from concourse import bacc
from concourse.masks import make_identity

F32 = mybir.dt.float32

# Problem constants
B, H, W = 512, 280, 280
KS = 28
HS = WS = 10
F = 16
OUT = 10
NCORES = 8
BLOC = 128      # images per core
NBANDS = 5      # bands per core (half image)
NPW = 10        # patches per band
NCHUNK = 7      # 112-pixel chunks per patch (4 rows x 28 cols each)
CK = 112        # contraction chunk size
BAND_W = KS * W  # 7840 elements per band per image


# DMA chunks per band, k-rows each. 14/14 keeps the per-DMA fixed cost
# low (10 x-DMAs; each 15,680 B/partition) — finer chunking measurably
# inflates per-queue DMA busy more than the smoother arrivals save.
CHUNK_K = [14, 14]              # k0-13 | k14-27
CHUNK_KOFF = [0, 14]
# transpose-group phases per band: (t_lo, t_hi, patch groups rel. to band)
TGROUPS = [
    (0, 3, [(0, 1), (2, 3), (4, 5), (6, 7), (8, 9)]),
    (3, 5, [(0, 1, 2, 3), (4, 5, 6, 7), (8, 9)]),
    (5, 7, [(0, 1, 2, 3), (4, 5, 6, 7), (8, 9)]),
]


def build_program(n_bands=NBANDS, n_pw=NPW, use_is_transpose=True, use_bf16=False):
    np_loc = n_bands * n_pw
    ng = (np_loc + 3) // 4
    WDT = mybir.dt.bfloat16 if use_bf16 else F32
    YB = 3                  # y PSUM bufs (max concurrently-open groups)

    nc = bacc.Bacc("TRN2")
    x_d = nc.dram_tensor("x", [BLOC, n_bands * BAND_W], F32, kind="ExternalInput")
    w_d = nc.dram_tensor("w", [CK, np_loc * NCHUNK * F], WDT, kind="ExternalInput")
    b_d = nc.dram_tensor("bias", [128, ng], F32, kind="ExternalInput")
    d_d = nc.dram_tensor("dec", [128, ng * OUT], WDT, kind="ExternalInput")
    o_d = nc.dram_tensor("out", [OUT, BLOC], F32, kind="ExternalOutput")

    with tile.TileContext(nc) as tc:
        with (
            tc.tile_pool(name="const", bufs=1) as constp,
            tc.tile_pool(name="xc0", bufs=3) as xc0p,
            tc.tile_pool(name="xc12", bufs=3) as xc12p,
            tc.tile_pool(name="xpm", bufs=2) as xpmp,
            tc.tile_pool(name="xt", bufs=4) as xtp,
            tc.tile_pool(name="xtps", bufs=4, space="PSUM") as xtpsp,
            tc.tile_pool(name="yps", bufs=YB, space="PSUM") as ypsp,
            tc.tile_pool(name="ops", bufs=1, space="PSUM") as opsp,
        ):
            ident = constp.tile([128, 128], WDT)
            make_identity(nc, ident[:])
            zero_sb = constp.tile([128, 128], F32)
            nc.gpsimd.memset(zero_sb[:], 0.0)
            w_sb = constp.tile([CK, np_loc * NCHUNK * F], WDT)
            bias_sb = constp.tile([128, ng], F32)
            dec_sb = constp.tile([128, ng * OUT], WDT)
            y_sb = constp.tile([128, ng * 128], WDT)
            out_ps = opsp.tile([OUT, BLOC], F32)

            chunk_tiles = {}
            xpm_tiles = {}

            def load_chunk(b, s):
                kc = CHUNK_K[s]
                pool = xc0p if s == 0 else xc12p
                t = pool.tile([128, kc * W], F32, name=f"x_c{min(s, 1)}")
                off = b * BAND_W + CHUNK_KOFF[s] * W
                nc.sync.dma_start(out=t[:], in_=x_d[:, off:off + kc * W])
                chunk_tiles[(b, s)] = t

            def im2col(b, s, k0, k1, eng):
                # reorder chunk rows [b, (k pw l)] -> patch-major region
                # of x_pm [b, (pw k l)], casting fp32 -> bf16. (k0, k1)
                # are local to the chunk.
                if b not in xpm_tiles:
                    xpm_tiles[b] = xpmp.tile([128, BAND_W], WDT, name="x_pm")
                t = xpm_tiles[b]
                dst4 = t[:].rearrange("b (pw k l) -> b pw k l", pw=n_pw, k=KS)
                src4 = chunk_tiles[(b, s)][:].rearrange(
                    "b (k pw l) -> b pw k l", k=CHUNK_K[s], pw=n_pw)
                kg = CHUNK_KOFF[s]
                dst = dst4[:, :, kg + k0:kg + k1, :]
                src = src4[:, :, k0:k1, :]
                if eng == "act":
                    nc.scalar.activation(
                        out=dst, in_=src,
                        func=mybir.ActivationFunctionType.Copy)
                else:
                    nc.vector.tensor_copy(dst, src)



            y_tiles = {}

            def emit_mms(group, t_lo, t_hi, xt):
                nt = t_hi - t_lo
                for idx, pp in enumerate(group):
                    G, q = pp // 4, pp % 4
                    if G not in y_tiles:
                        yt = ypsp.tile([128, 128], F32, name="y_ps")
                        if G < YB:
                            # clear stale/NaN PSUM so gaps are finite
                            nc.vector.tensor_copy(yt[:], zero_sb[:])
                        y_tiles[G] = yt
                    yt = y_tiles[G]
                    for t in range(t_lo, t_hi):
                        rhs = xt[:, (idx * nt + t - t_lo) * 128:
                                 (idx * nt + t - t_lo + 1) * 128]
                        nc.tensor.matmul(
                            yt[32 * q:32 * q + F, :],
                            w_sb[:, (pp * NCHUNK + t) * F:
                                 (pp * NCHUNK + t + 1) * F],
                            rhs,
                            start=(t == 0),
                            stop=(t == NCHUNK - 1),
                            tile_position=(0, 32 * q),
                        )
                    if t_hi == NCHUNK and (q == 3 or pp == np_loc - 1):
                        nc.scalar.activation(
                            out=y_sb[:, G * 128:(G + 1) * 128],
                            in_=yt[:],
                            func=mybir.ActivationFunctionType.Relu,
                            bias=bias_sb[:, G:G + 1],
                        )
                        y_tiles.pop(G)
                        # decoder accumulation inline: keeps it off the tail
                        nc.tensor.matmul(
                            out_ps[:],
                            dec_sb[:, G * OUT:(G + 1) * OUT],
                            y_sb[:, G * 128:(G + 1) * 128],
                            start=(G == 0),
                            stop=(G == ng - 1),
                        )

            def transposes(group, t_lo, t_hi, evac_eng):
                # transpose the group's patches for t-range into one PSUM
                # tile, then evacuate with a single DVE copy (amortizes
                # PSUM-access init). Uniform full-bank tiles [CK, 1024].
                band = group[0] // n_pw
                x_pm = xpm_tiles[band]
                nt = t_hi - t_lo
                ps = xtpsp.tile([CK, 1024], WDT, name="xt_ps")
                sb = xtp.tile([CK, 1024], WDT, name="xt")
                for idx, p in enumerate(group):
                    pw = p % n_pw
                    for t in range(t_lo, t_hi):
                        col = idx * nt + t - t_lo
                        dst = ps[:, col * 128:(col + 1) * 128]
                        src = x_pm[:, pw * 784 + t * CK: pw * 784 + (t + 1) * CK]
                        nc.tensor.transpose(dst, src, ident[:])
                ncols = len(group) * nt * 128
                if evac_eng == "act":
                    nc.scalar.activation(
                        out=sb[:, :ncols], in_=ps[:, :ncols],
                        func=mybir.ActivationFunctionType.Copy)
                else:
                    nc.vector.tensor_copy(sb[:, :ncols], ps[:, :ncols])
                return sb

            # depth-2 software pipeline: mains for group i are emitted after
            # the transposes of group i+2, giving the DVE evac two full
            # transpose groups of PE time to complete in
            pending = []

            def step(group, t_lo, t_hi, evac_eng="vec"):
                xt = transposes(group, t_lo, t_hi, evac_eng)
                pending.append((group, t_lo, t_hi, xt))
                if len(pending) > 2:
                    emit_mms(*pending.pop(0))

            WB = n_pw * NCHUNK * F  # w_sb columns per band

            # weights/bias/dec go on the second HWDGE ring (ACT) so they
            # never sit in front of x chunks in the sync-ring FIFO
            def load_w(b):
                nc.scalar.dma_start(
                    out=w_sb[:, b * WB:(b + 1) * WB],
                    in_=w_d[:, b * WB:(b + 1) * WB])

            def load_w_rest():
                nc.scalar.dma_start(
                    out=w_sb[:, WB:], in_=w_d[:, WB:])

            for band in range(n_bands):
                p0 = band * n_pw
                if band == 0:
                    load_chunk(0, 0)
                    load_w(0)
                    load_chunk(0, 1)
                    nc.scalar.dma_start(out=bias_sb[:], in_=b_d[:])
                    if n_bands > 1:
                        load_chunk(1, 0)
                        load_w_rest()
                        load_chunk(1, 1)
                    if n_bands > 2:
                        load_chunk(2, 0)
                        load_chunk(2, 1)
                    nc.scalar.dma_start(out=dec_sb[:], in_=d_d[:])
                    # band 0 startup: c0 feeds the first transpose group
                    im2col(0, 0, 0, 8, "act")
                    im2col(0, 0, 8, 14, "vec")
                # im2col slots: chunks arrive just-in-time; pieces are
                # sized to each t-group's row needs and split across
                # ACT+DVE the moment data lands, in slots that don't
                # head-block evacs PE is about to need.
                for gi, (t_lo, t_hi, groups) in enumerate(TGROUPS):
                    for si, g in enumerate(groups):
                        if gi == 0:
                            if si == 2 and band + 3 < n_bands:
                                load_chunk(band + 3, 0)
                                load_chunk(band + 3, 1)
                            elif si == 3:
                                # c1 rows k14-19 feed t3-4 (B1)
                                im2col(band, 1, 0, 4, "act")
                            elif si == 4:
                                im2col(band, 1, 4, 6, "vec")
                        elif gi == 1:
                            if si == 0:
                                # c1 rows k20-27 feed t5-6 (B2)
                                im2col(band, 1, 6, 11, "act")
                            elif si == 1:
                                im2col(band, 1, 11, 14, "vec")
                        else:
                            if si == 0 and band + 1 < n_bands:
                                im2col(band + 1, 0, 0, 5, "act")
                            elif si == 1 and band + 1 < n_bands:
                                im2col(band + 1, 0, 5, 10, "vec")
                            elif si == 2 and band + 1 < n_bands:
                                im2col(band + 1, 0, 10, 14, "act")
                        step(tuple(p0 + i for i in g), t_lo, t_hi)
            while pending:
                emit_mms(*pending.pop(0))

            out_sb = constp.tile([OUT, BLOC], F32)
            nc.vector.tensor_copy(out_sb[:], out_ps[:])
            nc.sync.dma_start(out=o_d[:], in_=out_sb[:])

    return nc


def stage_half(weight, bias, dec_w, h, n_bands=NBANDS, n_pw=NPW):
    """Host-side staging of weights/bias/decoder for image-half h (0 or 1)."""
    np_loc = n_bands * n_pw
    ng = (np_loc + 3) // 4
    weight = np.asarray(weight, np.float32)
    bias = np.asarray(bias, np.float32)
    dec_w = np.asarray(dec_w, np.float32)

    # w: (1600, 1, 28, 28) -> [f, ph, pw, k, l] -> chunks [d=(kk,l), (bl,pw,t,f)]
    w5 = weight.reshape(F, HS, WS, KS, KS)[:, n_bands * h:n_bands * h + n_bands]
    w6 = w5.reshape(F, n_bands, WS, NCHUNK, 4, KS)  # f bl pw t kk l
    wst = np.ascontiguousarray(
        np.transpose(w6, (4, 5, 1, 2, 3, 0))).reshape(CK, np_loc * NCHUNK * F)

    b5 = bias.reshape(F, HS, WS)[:, n_bands * h:n_bands * h + n_bands, :]
    b5 = b5.reshape(F, np_loc)
    bst = np.zeros((128, ng), np.float32)
    d5 = dec_w.reshape(OUT, F, HS, WS)[:, :, n_bands * h:n_bands * h + n_bands, :]
    d5 = d5.reshape(OUT, F, np_loc)
    dst_ = np.zeros((128, ng * OUT), np.float32)
    for pl in range(np_loc):
        G, q = pl // 4, pl % 4
        bst[32 * q:32 * q + F, G] = b5[:, pl]
        dst_[32 * q:32 * q + F, G * OUT:(G + 1) * OUT] = d5[:, :, pl].T
    return wst, bst, dst_


_cache = {}
USE_BF16 = True
USE_IS_TRANSPOSE = True


def _get_nc():
    key = ("nc", USE_BF16, USE_IS_TRANSPOSE)
    if key not in _cache:
        nc = build_program(use_is_transpose=USE_IS_TRANSPOSE, use_bf16=USE_BF16)
        nc.finalize()
        _cache[key] = nc
    return _cache[key]


def make_in_maps(x, weight, bias, dec_w):
    x = np.asarray(x, np.float32)
    stages = [stage_half(weight, bias, dec_w, h) for h in (0, 1)]
    in_maps = []
    for core in range(NCORES):
        bg, h = core // 2, core % 2
        xs = np.ascontiguousarray(
            x[bg * BLOC:(bg + 1) * BLOC, 0, 140 * h:140 * h + 140, :]
        ).reshape(BLOC, NBANDS * BAND_W)
        wst, bst, dst_ = stages[h]
        if USE_BF16:
            import ml_dtypes
            wst = wst.astype(ml_dtypes.bfloat16)
            dst_ = dst_.astype(ml_dtypes.bfloat16)
        in_maps.append({"x": xs, "w": wst, "bias": bst, "dec": dst_})
    return in_maps


def combine(results, dec_b):
    out = np.zeros((B, OUT), np.float32)
    for bg in range(4):
        part = results[2 * bg]["out"] + results[2 * bg + 1]["out"]  # (10, 128)
        out[bg * BLOC:(bg + 1) * BLOC] = part.T + np.asarray(dec_b, np.float32)
    return out


def _install_ntff_hook():
    """Provide the missing antenv.axon_hooks module so trace=True works
    under axon (replicates trn_boot._ntff_profile_via_ctypes)."""
    import contextlib
    import ctypes
    import types

    if "antenv.axon_hooks" in sys.modules:
        return
    so_path = "/opt/axon/libaxon_pjrt.so"
    holder = {}
    mod = types.ModuleType("antenv.axon_hooks")
    mod.set_axon_ntff_profile_hook = lambda h: holder.__setitem__("h", h)
    mod.get_axon_ntff_profile_hook = lambda: holder.get("h")
    sys.modules["antenv.axon_hooks"] = mod
    try:
        import antenv
        antenv.axon_hooks = mod
    except ImportError:
        pass

    lib = ctypes.CDLL(so_path)
    if not hasattr(lib, "axon_start_nrt_profile"):
        return
    lib.axon_start_nrt_profile.argtypes = [
        ctypes.POINTER(ctypes.c_int64), ctypes.c_size_t]
    lib.axon_start_nrt_profile.restype = ctypes.c_int64
    lib.axon_stop_nrt_profile.argtypes = [ctypes.c_char_p]
    lib.axon_stop_nrt_profile.restype = ctypes.c_int64

    @contextlib.contextmanager
    def _hook(output_dir, device_ids):
        import jax
        jax.devices()
        if device_ids:
            ids = (ctypes.c_int64 * len(device_ids))(*device_ids)
            rc = lib.axon_start_nrt_profile(ids, len(device_ids))
        else:
            rc = lib.axon_start_nrt_profile(None, 0)
        if rc != 0:
            raise RuntimeError(f"axon_start_nrt_profile rc={rc}")
        try:
            yield
        finally:
            n = lib.axon_stop_nrt_profile(str(output_dir).encode())
            print(f"profile: {n} file(s) written to {output_dir}")

    mod.set_axon_ntff_profile_hook(_hook)


def run(x, weight, bias, dec_w, dec_b, trace=False):
    from concourse import bass_utils
    from concourse.bass_utils import run_bass_kernel_spmd

    if trace:
        _install_ntff_hook()
        # artifact upload needs a bucket that doesn't exist here
        bass_utils.upload_artifacts = lambda tmpdir: tmpdir

    nc = _get_nc()
    in_maps = make_in_maps(x, weight, bias, dec_w)
    r = run_bass_kernel_spmd(nc, in_maps, list(range(NCORES)), trace=trace)
    return combine(r.results, dec_b), r


def kernel(x, weight, bias, dec_w, dec_b):
    out, _ = run(x, weight, bias, dec_w, dec_b, trace=False)
    return out



# revision 43
# speedup vs baseline: 1.0661x; 1.0661x over previous
"""Trainium2 Bass kernel for nn_LCN (locally-connected network).

Computation (see module docstring math):
  x: (512, 1, 280, 280) -> non-overlapping 28x28 patches (10x10 grid, P=100)
  y[b, f, p] = sum_{k,l} x[b, 28ph+k, 28pw+l] * w[f*100+p, 0, k, l]
  y = relu(y + bias[f*100+p]);  out = y_flat @ dec_w.T + dec_b   (j = f*100 + p)

Sharding: 8 cores = 4 batch groups x 2 image halves (rows 0..139 | 140..279).
Each core: 128 images, 5 bands (28 rows each), 50 patches.
Per core pipeline:
  - DMA band [128b, 7840] (fp32, contiguous in HBM)
  - PE transpose x chunks [128b, 112pix] -> PSUM [112, 128] (identity matmul)
  - DVE/ACT evacuate PSUM -> SBUF
  - per patch: 7 accumulating matmuls lhsT=w[112,16], rhs=xT[112,128] -> y PSUM
    (4 patches per PSUM tile at partition offsets 0/32/64/96)
  - ACT: relu(y + bias) -> y_sb  [j on partitions, gappy layout]
  - decoder: 13 accumulating matmuls lhsT=dec[128,10] (zeros in gaps) -> out [10,128]
Host sums the two half-image partial decoder outputs and adds dec_b.
"""

import sys

import numpy as np

for _p in ("/opt/trn_rl_repo", "/opt/trn_rl_repo/concourse"):
    if _p not in sys.path:
        sys.path.insert(0, _p)

import concourse.bass as bass
import concourse.mybir as mybir
import concourse.tile as tile
from concourse import bacc
from concourse.masks import make_identity

F32 = mybir.dt.float32

# Problem constants
B, H, W = 512, 280, 280
KS = 28
HS = WS = 10
F = 16
OUT = 10
NCORES = 8
BLOC = 128      # images per core
NBANDS = 5      # bands per core (half image)
NPW = 10        # patches per band
NCHUNK = 7      # 112-pixel chunks per patch (4 rows x 28 cols each)
CK = 112        # contraction chunk size
BAND_W = KS * W  # 7840 elements per band per image


# DMA chunks per band, k-rows each. 14/14 keeps the per-DMA fixed cost
# low (10 x-DMAs; each 15,680 B/partition) — finer chunking measurably
# inflates per-queue DMA busy more than the smoother arrivals save.
CHUNK_K = [14, 14]              # k0-13 | k14-27
CHUNK_KOFF = [0, 14]
# transpose-group phases per band: (t_lo, t_hi, patch groups rel. to band)
TGROUPS = [
    (0, 3, [(0, 1), (2, 3), (4, 5), (6, 7), (8, 9)]),
    (3, 5, [(0, 1, 2, 3), (4, 5, 6, 7), (8, 9)]),
    (5, 7, [(0, 1, 2, 3), (4, 5, 6, 7), (8, 9)]),
]


def build_program(n_bands=NBANDS, n_pw=NPW, use_is_transpose=True, use_bf16=False):
    np_loc = n_bands * n_pw
    ng = (np_loc + 3) // 4
    WDT = mybir.dt.bfloat16 if use_bf16 else F32
    YB = 3                  # y PSUM bufs (max concurrently-open groups)

    nc = bacc.Bacc("TRN2")
    x_d = nc.dram_tensor("x", [BLOC, n_bands * BAND_W], F32, kind="ExternalInput")
    w_d = nc.dram_tensor("w", [CK, np_loc * NCHUNK * F], WDT, kind="ExternalInput")
    b_d = nc.dram_tensor("bias", [128, ng], F32, kind="ExternalInput")
    d_d = nc.dram_tensor("dec", [128, ng * OUT], WDT, kind="ExternalInput")
    o_d = nc.dram_tensor("out", [OUT, BLOC], F32, kind="ExternalOutput")

    with tile.TileContext(nc) as tc:
        with (
            tc.tile_pool(name="const", bufs=1) as constp,
            tc.tile_pool(name="xc0", bufs=3) as xc0p,
            tc.tile_pool(name="xc12", bufs=3) as xc12p,
            tc.tile_pool(name="xpm", bufs=2) as xpmp,
            tc.tile_pool(name="xt", bufs=4) as xtp,
            tc.tile_pool(name="xtps", bufs=4, space="PSUM") as xtpsp,
            tc.tile_pool(name="yps", bufs=YB, space="PSUM") as ypsp,
            tc.tile_pool(name="ops", bufs=1, space="PSUM") as opsp,
        ):
            ident = constp.tile([128, 128], WDT)
            make_identity(nc, ident[:])
            zero_sb = constp.tile([128, 128], F32)
            nc.gpsimd.memset(zero_sb[:], 0.0)
            w_sb = constp.tile([CK, np_loc * NCHUNK * F], WDT)
            bias_sb = constp.tile([128, ng], F32)
            dec_sb = constp.tile([128, ng * OUT], WDT)
            y_sb = constp.tile([128, ng * 128], WDT)
            out_ps = opsp.tile([OUT, BLOC], F32)

            chunk_tiles = {}
            xpm_tiles = {}

            def load_chunk(b, s):
                kc = CHUNK_K[s]
                pool = xc0p if s == 0 else xc12p
                t = pool.tile([128, kc * W], F32, name=f"x_c{min(s, 1)}")
                off = b * BAND_W + CHUNK_KOFF[s] * W
                nc.sync.dma_start(out=t[:], in_=x_d[:, off:off + kc * W])
                chunk_tiles[(b, s)] = t

            def im2col(b, s, k0, k1, eng):
                # reorder chunk rows [b, (k pw l)] -> patch-major region
                # of x_pm [b, (pw k l)], casting fp32 -> bf16. (k0, k1)
                # are local to the chunk.
                if b not in xpm_tiles:
                    xpm_tiles[b] = xpmp.tile([128, BAND_W], WDT, name="x_pm")
                t = xpm_tiles[b]
                dst4 = t[:].rearrange("b (pw k l) -> b pw k l", pw=n_pw, k=KS)
                src4 = chunk_tiles[(b, s)][:].rearrange(
                    "b (k pw l) -> b pw k l", k=CHUNK_K[s], pw=n_pw)
                kg = CHUNK_KOFF[s]
                dst = dst4[:, :, kg + k0:kg + k1, :]
                src = src4[:, :, k0:k1, :]
                if eng == "act":
                    nc.scalar.activation(
                        out=dst, in_=src,
                        func=mybir.ActivationFunctionType.Copy)
                else:
                    nc.vector.tensor_copy(dst, src)



            y_tiles = {}

            def emit_mms(group, t_lo, t_hi, xt):
                nt = t_hi - t_lo
                for idx, pp in enumerate(group):
                    G, q = pp // 4, pp % 4
                    if G not in y_tiles:
                        yt = ypsp.tile([128, 128], F32, name="y_ps")
                        if G < YB:
                            # clear stale/NaN PSUM so gaps are finite
                            nc.vector.tensor_copy(yt[:], zero_sb[:])
                        y_tiles[G] = yt
                    yt = y_tiles[G]
                    for t in range(t_lo, t_hi):
                        rhs = xt[:, (idx * nt + t - t_lo) * 128:
                                 (idx * nt + t - t_lo + 1) * 128]
                        nc.tensor.matmul(
                            yt[32 * q:32 * q + F, :],
                            w_sb[:, (pp * NCHUNK + t) * F:
                                 (pp * NCHUNK + t + 1) * F],
                            rhs,
                            start=(t == 0),
                            stop=(t == NCHUNK - 1),
                            tile_position=(0, 32 * q),
                        )
                    if t_hi == NCHUNK and (q == 3 or pp == np_loc - 1):
                        nc.scalar.activation(
                            out=y_sb[:, G * 128:(G + 1) * 128],
                            in_=yt[:],
                            func=mybir.ActivationFunctionType.Relu,
                            bias=bias_sb[:, G:G + 1],
                        )
                        y_tiles.pop(G)
                        # decoder accumulation inline: keeps it off the tail
                        nc.tensor.matmul(
                            out_ps[:],
                            dec_sb[:, G * OUT:(G + 1) * OUT],
                            y_sb[:, G * 128:(G + 1) * 128],
                            start=(G == 0),
                            stop=(G == ng - 1),
                        )

            def transposes(group, t_lo, t_hi, evac_eng):
                # transpose the group's patches for t-range into one PSUM
                # tile, then evacuate with a single DVE copy (amortizes
                # PSUM-access init). Uniform full-bank tiles [CK, 1024].
                band = group[0] // n_pw
                x_pm = xpm_tiles[band]
                nt = t_hi - t_lo
                ps = xtpsp.tile([CK, 1024], WDT, name="xt_ps")
                sb = xtp.tile([CK, 1024], WDT, name="xt")
                for idx, p in enumerate(group):
                    pw = p % n_pw
                    for t in range(t_lo, t_hi):
                        col = idx * nt + t - t_lo
                        dst = ps[:, col * 128:(col + 1) * 128]
                        src = x_pm[:, pw * 784 + t * CK: pw * 784 + (t + 1) * CK]
                        nc.tensor.transpose(dst, src, ident[:])
                ncols = len(group) * nt * 128
                if evac_eng == "act":
                    nc.scalar.activation(
                        out=sb[:, :ncols], in_=ps[:, :ncols],
                        func=mybir.ActivationFunctionType.Copy)
                else:
                    nc.vector.tensor_copy(sb[:, :ncols], ps[:, :ncols])
                return sb

            # depth-2 software pipeline: mains for group i are emitted after
            # the transposes of group i+2, giving the DVE evac two full
            # transpose groups of PE time to complete in
            pending = []

            def step(group, t_lo, t_hi, evac_eng="vec"):
                xt = transposes(group, t_lo, t_hi, evac_eng)
                pending.append((group, t_lo, t_hi, xt))
                if len(pending) > 2:
                    emit_mms(*pending.pop(0))

            WB = n_pw * NCHUNK * F  # w_sb columns per band

            # weights/bias/dec go on the second HWDGE ring (ACT) so they
            # never sit in front of x chunks in the sync-ring FIFO
            def load_w(b):
                nc.sync.dma_start(
                    out=w_sb[:, b * WB:(b + 1) * WB],
                    in_=w_d[:, b * WB:(b + 1) * WB])

            def load_w_rest():
                nc.sync.dma_start(
                    out=w_sb[:, WB:], in_=w_d[:, WB:])

            for band in range(n_bands):
                p0 = band * n_pw
                if band == 0:
                    load_chunk(0, 0)
                    load_w(0)
                    load_chunk(0, 1)
                    nc.sync.dma_start(out=bias_sb[:], in_=b_d[:])
                    if n_bands > 1:
                        load_chunk(1, 0)
                        load_w_rest()
                        load_chunk(1, 1)
                    if n_bands > 2:
                        load_chunk(2, 0)
                        load_chunk(2, 1)
                    nc.sync.dma_start(out=dec_sb[:], in_=d_d[:])
                    # band 0 startup: c0 feeds the first transpose group
                    im2col(0, 0, 0, 8, "act")
                    im2col(0, 0, 8, 14, "vec")
                # im2col slots: chunks arrive just-in-time; pieces are
                # sized to each t-group's row needs and split across
                # ACT+DVE the moment data lands, in slots that don't
                # head-block evacs PE is about to need.
                for gi, (t_lo, t_hi, groups) in enumerate(TGROUPS):
                    for si, g in enumerate(groups):
                        if gi == 0:
                            if si == 2 and band + 3 < n_bands:
                                load_chunk(band + 3, 0)
                                load_chunk(band + 3, 1)
                            elif si == 3:
                                # c1 rows k14-19 feed t3-4 (B1)
                                im2col(band, 1, 0, 4, "act")
                            elif si == 4:
                                im2col(band, 1, 4, 6, "vec")
                        elif gi == 1:
                            if si == 0:
                                # c1 rows k20-27 feed t5-6 (B2)
                                im2col(band, 1, 6, 11, "act")
                            elif si == 1:
                                im2col(band, 1, 11, 14, "vec")
                        else:
                            if si == 0 and band + 1 < n_bands:
                                im2col(band + 1, 0, 0, 5, "act")
                            elif si == 1 and band + 1 < n_bands:
                                im2col(band + 1, 0, 5, 10, "vec")
                            elif si == 2 and band + 1 < n_bands:
                                im2col(band + 1, 0, 10, 14, "act")
                        step(tuple(p0 + i for i in g), t_lo, t_hi)
            while pending:
                emit_mms(*pending.pop(0))

            out_sb = constp.tile([OUT, BLOC], F32)
            nc.vector.tensor_copy(out_sb[:], out_ps[:])
            nc.sync.dma_start(out=o_d[:], in_=out_sb[:])

    return nc


def stage_half(weight, bias, dec_w, h, n_bands=NBANDS, n_pw=NPW):
    """Host-side staging of weights/bias/decoder for image-half h (0 or 1)."""
    np_loc = n_bands * n_pw
    ng = (np_loc + 3) // 4
    weight = np.asarray(weight, np.float32)
    bias = np.asarray(bias, np.float32)
    dec_w = np.asarray(dec_w, np.float32)

    # w: (1600, 1, 28, 28) -> [f, ph, pw, k, l] -> chunks [d=(kk,l), (bl,pw,t,f)]
    w5 = weight.reshape(F, HS, WS, KS, KS)[:, n_bands * h:n_bands * h + n_bands]
    w6 = w5.reshape(F, n_bands, WS, NCHUNK, 4, KS)  # f bl pw t kk l
    wst = np.ascontiguousarray(
        np.transpose(w6, (4, 5, 1, 2, 3, 0))).reshape(CK, np_loc * NCHUNK * F)

    b5 = bias.reshape(F, HS, WS)[:, n_bands * h:n_bands * h + n_bands, :]
    b5 = b5.reshape(F, np_loc)
    bst = np.zeros((128, ng), np.float32)
    d5 = dec_w.reshape(OUT, F, HS, WS)[:, :, n_bands * h:n_bands * h + n_bands, :]
    d5 = d5.reshape(OUT, F, np_loc)
    dst_ = np.zeros((128, ng * OUT), np.float32)
    for pl in range(np_loc):
        G, q = pl // 4, pl % 4
        bst[32 * q:32 * q + F, G] = b5[:, pl]
        dst_[32 * q:32 * q + F, G * OUT:(G + 1) * OUT] = d5[:, :, pl].T
    return wst, bst, dst_


_cache = {}
USE_BF16 = True
USE_IS_TRANSPOSE = True


def _get_nc():
    key = ("nc", USE_BF16, USE_IS_TRANSPOSE)
    if key not in _cache:
        nc = build_program(use_is_transpose=USE_IS_TRANSPOSE, use_bf16=USE_BF16)
        nc.finalize()
        _cache[key] = nc
    return _cache[key]


def make_in_maps(x, weight, bias, dec_w):
    x = np.asarray(x, np.float32)
    stages = [stage_half(weight, bias, dec_w, h) for h in (0, 1)]
    in_maps = []
    for core in range(NCORES):
        bg, h = core // 2, core % 2
        xs = np.ascontiguousarray(
            x[bg * BLOC:(bg + 1) * BLOC, 0, 140 * h:140 * h + 140, :]
        ).reshape(BLOC, NBANDS * BAND_W)
        wst, bst, dst_ = stages[h]
        if USE_BF16:
            import ml_dtypes
            wst = wst.astype(ml_dtypes.bfloat16)
            dst_ = dst_.astype(ml_dtypes.bfloat16)
        in_maps.append({"x": xs, "w": wst, "bias": bst, "dec": dst_})
    return in_maps


def combine(results, dec_b):
    out = np.zeros((B, OUT), np.float32)
    for bg in range(4):
        part = results[2 * bg]["out"] + results[2 * bg + 1]["out"]  # (10, 128)
        out[bg * BLOC:(bg + 1) * BLOC] = part.T + np.asarray(dec_b, np.float32)
    return out


def _install_ntff_hook():
    """Provide the missing antenv.axon_hooks module so trace=True works
    under axon (replicates trn_boot._ntff_profile_via_ctypes)."""
    import contextlib
    import ctypes
    import types

    if "antenv.axon_hooks" in sys.modules:
        return
    so_path = "/opt/axon/libaxon_pjrt.so"
    holder = {}
    mod = types.ModuleType("antenv.axon_hooks")
    mod.set_axon_ntff_profile_hook = lambda h: holder.__setitem__("h", h)
    mod.get_axon_ntff_profile_hook = lambda: holder.get("h")
    sys.modules["antenv.axon_hooks"] = mod
    try:
        import antenv
        antenv.axon_hooks = mod
    except ImportError:
        pass

    lib = ctypes.CDLL(so_path)
    if not hasattr(lib, "axon_start_nrt_profile"):
        return
    lib.axon_start_nrt_profile.argtypes = [
        ctypes.POINTER(ctypes.c_int64), ctypes.c_size_t]
    lib.axon_start_nrt_profile.restype = ctypes.c_int64
    lib.axon_stop_nrt_profile.argtypes = [ctypes.c_char_p]
    lib.axon_stop_nrt_profile.restype = ctypes.c_int64

    @contextlib.contextmanager
    def _hook(output_dir, device_ids):
        import jax
        jax.devices()
        if device_ids:
            ids = (ctypes.c_int64 * len(device_ids))(*device_ids)
            rc = lib.axon_start_nrt_profile(ids, len(device_ids))
        else:
            rc = lib.axon_start_nrt_profile(None, 0)
        if rc != 0:
            raise RuntimeError(f"axon_start_nrt_profile rc={rc}")
        try:
            yield
        finally:
            n = lib.axon_stop_nrt_profile(str(output_dir).encode())
            print(f"profile: {n} file(s) written to {output_dir}")

    mod.set_axon_ntff_profile_hook(_hook)


def run(x, weight, bias, dec_w, dec_b, trace=False):
    from concourse import bass_utils
    from concourse.bass_utils import run_bass_kernel_spmd

    if trace:
        _install_ntff_hook()
        # artifact upload needs a bucket that doesn't exist here
        bass_utils.upload_artifacts = lambda tmpdir: tmpdir

    nc = _get_nc()
    in_maps = make_in_maps(x, weight, bias, dec_w)
    r = run_bass_kernel_spmd(nc, in_maps, list(range(NCORES)), trace=trace)
    return combine(r.results, dec_b), r


def kernel(x, weight, bias, dec_w, dec_b):
    out, _ = run(x, weight, bias, dec_w, dec_b, trace=False)
    return out



# revision 47
# speedup vs baseline: 1.0861x; 1.0188x over previous
"""Trainium2 Bass kernel for nn_LCN (locally-connected network).

Computation (see module docstring math):
  x: (512, 1, 280, 280) -> non-overlapping 28x28 patches (10x10 grid, P=100)
  y[b, f, p] = sum_{k,l} x[b, 28ph+k, 28pw+l] * w[f*100+p, 0, k, l]
  y = relu(y + bias[f*100+p]);  out = y_flat @ dec_w.T + dec_b   (j = f*100 + p)

Sharding: 8 cores = 4 batch groups x 2 image halves (rows 0..139 | 140..279).
Each core: 128 images, 5 bands (28 rows each), 50 patches.
Per core pipeline:
  - DMA band [128b, 7840] (fp32, contiguous in HBM)
  - PE transpose x chunks [128b, 112pix] -> PSUM [112, 128] (identity matmul)
  - DVE/ACT evacuate PSUM -> SBUF
  - per patch: 7 accumulating matmuls lhsT=w[112,16], rhs=xT[112,128] -> y PSUM
    (4 patches per PSUM tile at partition offsets 0/32/64/96)
  - ACT: relu(y + bias) -> y_sb  [j on partitions, gappy layout]
  - decoder: 13 accumulating matmuls lhsT=dec[128,10] (zeros in gaps) -> out [10,128]
Host sums the two half-image partial decoder outputs and adds dec_b.
"""

import sys

import numpy as np

for _p in ("/opt/trn_rl_repo", "/opt/trn_rl_repo/concourse"):
    if _p not in sys.path:
        sys.path.insert(0, _p)

import concourse.bass as bass
import concourse.mybir as mybir
import concourse.tile as tile
from concourse import bacc
from concourse.masks import make_identity

F32 = mybir.dt.float32

# Problem constants
B, H, W = 512, 280, 280
KS = 28
HS = WS = 10
F = 16
OUT = 10
NCORES = 8
BLOC = 128      # images per core
NBANDS = 5      # bands per core (half image)
NPW = 10        # patches per band
NCHUNK = 7      # 112-pixel chunks per patch (4 rows x 28 cols each)
CK = 112        # contraction chunk size
BAND_W = KS * W  # 7840 elements per band per image


# DMA chunks per band, k-rows each. 14/14 keeps the per-DMA fixed cost
# low (10 x-DMAs; each 15,680 B/partition) — finer chunking measurably
# inflates per-queue DMA busy more than the smoother arrivals save.
CHUNK_K = [14, 14]              # k0-13 | k14-27
CHUNK_KOFF = [0, 14]
# transpose-group phases per band: (t_lo, t_hi, patch groups rel. to band)
TGROUPS = [
    (0, 3, [(0, 1), (2, 3), (4, 5), (6, 7), (8, 9)]),
    (3, 5, [(0, 1, 2, 3), (4, 5, 6, 7), (8, 9)]),
    (5, 7, [(0, 1, 2, 3), (4, 5, 6, 7), (8, 9)]),
]


def build_program(n_bands=NBANDS, n_pw=NPW, use_is_transpose=True, use_bf16=False):
    np_loc = n_bands * n_pw
    ng = (np_loc + 3) // 4
    WDT = mybir.dt.bfloat16 if use_bf16 else F32
    YB = 3                  # y PSUM bufs (max concurrently-open groups)

    nc = bacc.Bacc("TRN2")
    x_d = nc.dram_tensor("x", [BLOC, n_bands * BAND_W], F32, kind="ExternalInput")
    w_d = nc.dram_tensor("w", [CK, np_loc * NCHUNK * F], WDT, kind="ExternalInput")
    b_d = nc.dram_tensor("bias", [128, ng], F32, kind="ExternalInput")
    d_d = nc.dram_tensor("dec", [128, ng * OUT], WDT, kind="ExternalInput")
    o_d = nc.dram_tensor("out", [OUT, BLOC], F32, kind="ExternalOutput")

    with tile.TileContext(nc) as tc:
        with (
            tc.tile_pool(name="const", bufs=1) as constp,
            tc.tile_pool(name="xc0", bufs=3) as xc0p,
            tc.tile_pool(name="xc12", bufs=3) as xc12p,
            tc.tile_pool(name="xpm", bufs=2) as xpmp,
            tc.tile_pool(name="xt", bufs=4) as xtp,
            tc.tile_pool(name="xtps", bufs=4, space="PSUM") as xtpsp,
            tc.tile_pool(name="yps", bufs=YB, space="PSUM") as ypsp,
            tc.tile_pool(name="ops", bufs=1, space="PSUM") as opsp,
        ):
            ident = constp.tile([128, 128], WDT)
            make_identity(nc, ident[:])
            zero_sb = constp.tile([128, 128], F32)
            nc.gpsimd.memset(zero_sb[:], 0.0)
            w_sb = constp.tile([CK, np_loc * NCHUNK * F], WDT)
            bias_sb = constp.tile([128, ng], F32)
            dec_sb = constp.tile([128, ng * OUT], WDT)
            y_sb = constp.tile([128, ng * 128], WDT)
            out_ps = opsp.tile([OUT, BLOC], F32)

            chunk_tiles = {}
            xpm_tiles = {}

            def load_chunk(b, s):
                kc = CHUNK_K[s]
                pool = xc0p if s == 0 else xc12p
                t = pool.tile([128, kc * W], F32, name=f"x_c{min(s, 1)}")
                off = b * BAND_W + CHUNK_KOFF[s] * W
                nc.sync.dma_start(out=t[:], in_=x_d[:, off:off + kc * W])
                chunk_tiles[(b, s)] = t

            def im2col(b, s, pw0, pw1, eng):
                # reorder chunk columns for patches [pw0, pw1) -> patch-
                # major region of x_pm [b, (pw k l)], casting fp32 -> bf16.
                # pw-wise pieces let each PE step gate on one small copy
                # instead of a whole chunk's reorder.
                if b not in xpm_tiles:
                    xpm_tiles[b] = xpmp.tile([128, BAND_W], WDT, name="x_pm")
                t = xpm_tiles[b]
                dst4 = t[:].rearrange("b (pw k l) -> b pw k l", pw=n_pw, k=KS)
                src4 = chunk_tiles[(b, s)][:].rearrange(
                    "b (k pw l) -> b pw k l", k=CHUNK_K[s], pw=n_pw)
                kg = CHUNK_KOFF[s]
                dst = dst4[:, pw0:pw1, kg:kg + CHUNK_K[s], :]
                src = src4[:, pw0:pw1, :, :]
                if eng == "act":
                    nc.scalar.activation(
                        out=dst, in_=src,
                        func=mybir.ActivationFunctionType.Copy)
                else:
                    nc.vector.tensor_copy(dst, src)



            y_tiles = {}

            def emit_mms(group, t_lo, t_hi, xt):
                nt = t_hi - t_lo
                for idx, pp in enumerate(group):
                    G, q = pp // 4, pp % 4
                    if G not in y_tiles:
                        yt = ypsp.tile([128, 128], F32, name="y_ps")
                        if G < YB:
                            # clear stale/NaN PSUM so gaps are finite
                            nc.vector.tensor_copy(yt[:], zero_sb[:])
                        y_tiles[G] = yt
                    yt = y_tiles[G]
                    for t in range(t_lo, t_hi):
                        rhs = xt[:, (idx * nt + t - t_lo) * 128:
                                 (idx * nt + t - t_lo + 1) * 128]
                        nc.tensor.matmul(
                            yt[32 * q:32 * q + F, :],
                            w_sb[:, (pp * NCHUNK + t) * F:
                                 (pp * NCHUNK + t + 1) * F],
                            rhs,
                            start=(t == 0),
                            stop=(t == NCHUNK - 1),
                            tile_position=(0, 32 * q),
                        )
                    if t_hi == NCHUNK and (q == 3 or pp == np_loc - 1):
                        nc.scalar.activation(
                            out=y_sb[:, G * 128:(G + 1) * 128],
                            in_=yt[:],
                            func=mybir.ActivationFunctionType.Relu,
                            bias=bias_sb[:, G:G + 1],
                        )
                        y_tiles.pop(G)
                        # decoder accumulation inline: keeps it off the tail
                        nc.tensor.matmul(
                            out_ps[:],
                            dec_sb[:, G * OUT:(G + 1) * OUT],
                            y_sb[:, G * 128:(G + 1) * 128],
                            start=(G == 0),
                            stop=(G == ng - 1),
                        )

            def transposes(group, t_lo, t_hi, evac_eng):
                # transpose the group's patches for t-range into one PSUM
                # tile, then evacuate with a single DVE copy (amortizes
                # PSUM-access init). Uniform full-bank tiles [CK, 1024].
                band = group[0] // n_pw
                x_pm = xpm_tiles[band]
                nt = t_hi - t_lo
                ps = xtpsp.tile([CK, 1024], WDT, name="xt_ps")
                sb = xtp.tile([CK, 1024], WDT, name="xt")
                for idx, p in enumerate(group):
                    pw = p % n_pw
                    for t in range(t_lo, t_hi):
                        col = idx * nt + t - t_lo
                        dst = ps[:, col * 128:(col + 1) * 128]
                        src = x_pm[:, pw * 784 + t * CK: pw * 784 + (t + 1) * CK]
                        nc.tensor.transpose(dst, src, ident[:])
                ncols = len(group) * nt * 128
                if evac_eng == "act":
                    nc.scalar.activation(
                        out=sb[:, :ncols], in_=ps[:, :ncols],
                        func=mybir.ActivationFunctionType.Copy)
                else:
                    nc.vector.tensor_copy(sb[:, :ncols], ps[:, :ncols])
                return sb

            # depth-2 software pipeline: mains for group i are emitted after
            # the transposes of group i+2, giving the DVE evac two full
            # transpose groups of PE time to complete in
            pending = []

            def step(group, t_lo, t_hi, evac_eng="vec"):
                xt = transposes(group, t_lo, t_hi, evac_eng)
                pending.append((group, t_lo, t_hi, xt))
                if len(pending) > 2:
                    emit_mms(*pending.pop(0))

            WB = n_pw * NCHUNK * F  # w_sb columns per band

            # weights/bias/dec go on the second HWDGE ring (ACT) so they
            # never sit in front of x chunks in the sync-ring FIFO
            def load_w(b):
                nc.sync.dma_start(
                    out=w_sb[:, b * WB:(b + 1) * WB],
                    in_=w_d[:, b * WB:(b + 1) * WB])

            def load_w_rest():
                nc.sync.dma_start(
                    out=w_sb[:, WB:], in_=w_d[:, WB:])

            for band in range(n_bands):
                p0 = band * n_pw
                if band == 0:
                    load_chunk(0, 0)
                    load_w(0)
                    load_chunk(0, 1)
                    nc.sync.dma_start(out=bias_sb[:], in_=b_d[:])
                    if n_bands > 1:
                        load_chunk(1, 0)
                        load_w_rest()
                        load_chunk(1, 1)
                    if n_bands > 2:
                        load_chunk(2, 0)
                        load_chunk(2, 1)
                    nc.sync.dma_start(out=dec_sb[:], in_=d_d[:])
                    # band 0 startup: c0 pieces feed the first A steps
                    im2col(0, 0, 0, 2, "act")
                    im2col(0, 0, 2, 4, "vec")
                    im2col(0, 0, 4, 6, "act")
                # im2col slots: chunks arrive just-in-time; pw-pair pieces
                # gate each PE step on a ~0.7us copy instead of a whole
                # chunk reorder, in slots that don't head-block evacs PE
                # is about to need.
                for gi, (t_lo, t_hi, groups) in enumerate(TGROUPS):
                    for si, g in enumerate(groups):
                        if gi == 0:
                            if si == 0:
                                im2col(band, 0, 6, 8, "vec")
                            elif si == 1:
                                im2col(band, 0, 8, 10, "act")
                            elif si == 2:
                                if band + 3 < n_bands:
                                    load_chunk(band + 3, 0)
                                    load_chunk(band + 3, 1)
                                im2col(band, 1, 0, 2, "act")
                            elif si == 3:
                                im2col(band, 1, 2, 4, "vec")
                            elif si == 4:
                                im2col(band, 1, 4, 6, "act")
                        elif gi == 1:
                            if si == 0:
                                im2col(band, 1, 6, 8, "act")
                            elif si == 1:
                                im2col(band, 1, 8, 10, "act")
                        else:
                            if si == 0 and band + 1 < n_bands:
                                im2col(band + 1, 0, 0, 2, "act")
                            elif si == 1 and band + 1 < n_bands:
                                im2col(band + 1, 0, 2, 4, "vec")
                            elif si == 2 and band + 1 < n_bands:
                                im2col(band + 1, 0, 4, 6, "act")
                        step(tuple(p0 + i for i in g), t_lo, t_hi)
            while pending:
                emit_mms(*pending.pop(0))

            out_sb = constp.tile([OUT, BLOC], F32)
            nc.vector.tensor_copy(out_sb[:], out_ps[:])
            nc.sync.dma_start(out=o_d[:], in_=out_sb[:])

    return nc


def stage_half(weight, bias, dec_w, h, n_bands=NBANDS, n_pw=NPW):
    """Host-side staging of weights/bias/decoder for image-half h (0 or 1)."""
    np_loc = n_bands * n_pw
    ng = (np_loc + 3) // 4
    weight = np.asarray(weight, np.float32)
    bias = np.asarray(bias, np.float32)
    dec_w = np.asarray(dec_w, np.float32)

    # w: (1600, 1, 28, 28) -> [f, ph, pw, k, l] -> chunks [d=(kk,l), (bl,pw,t,f)]
    w5 = weight.reshape(F, HS, WS, KS, KS)[:, n_bands * h:n_bands * h + n_bands]
    w6 = w5.reshape(F, n_bands, WS, NCHUNK, 4, KS)  # f bl pw t kk l
    wst = np.ascontiguousarray(
        np.transpose(w6, (4, 5, 1, 2, 3, 0))).reshape(CK, np_loc * NCHUNK * F)

    b5 = bias.reshape(F, HS, WS)[:, n_bands * h:n_bands * h + n_bands, :]
    b5 = b5.reshape(F, np_loc)
    bst = np.zeros((128, ng), np.float32)
    d5 = dec_w.reshape(OUT, F, HS, WS)[:, :, n_bands * h:n_bands * h + n_bands, :]
    d5 = d5.reshape(OUT, F, np_loc)
    dst_ = np.zeros((128, ng * OUT), np.float32)
    for pl in range(np_loc):
        G, q = pl // 4, pl % 4
        bst[32 * q:32 * q + F, G] = b5[:, pl]
        dst_[32 * q:32 * q + F, G * OUT:(G + 1) * OUT] = d5[:, :, pl].T
    return wst, bst, dst_


_cache = {}
USE_BF16 = True
USE_IS_TRANSPOSE = True


def _get_nc():
    key = ("nc", USE_BF16, USE_IS_TRANSPOSE)
    if key not in _cache:
        nc = build_program(use_is_transpose=USE_IS_TRANSPOSE, use_bf16=USE_BF16)
        nc.finalize()
        _cache[key] = nc
    return _cache[key]


def make_in_maps(x, weight, bias, dec_w):
    x = np.asarray(x, np.float32)
    stages = [stage_half(weight, bias, dec_w, h) for h in (0, 1)]
    in_maps = []
    for core in range(NCORES):
        bg, h = core // 2, core % 2
        xs = np.ascontiguousarray(
            x[bg * BLOC:(bg + 1) * BLOC, 0, 140 * h:140 * h + 140, :]
        ).reshape(BLOC, NBANDS * BAND_W)
        wst, bst, dst_ = stages[h]
        if USE_BF16:
            import ml_dtypes
            wst = wst.astype(ml_dtypes.bfloat16)
            dst_ = dst_.astype(ml_dtypes.bfloat16)
        in_maps.append({"x": xs, "w": wst, "bias": bst, "dec": dst_})
    return in_maps


def combine(results, dec_b):
    out = np.zeros((B, OUT), np.float32)
    for bg in range(4):
        part = results[2 * bg]["out"] + results[2 * bg + 1]["out"]  # (10, 128)
        out[bg * BLOC:(bg + 1) * BLOC] = part.T + np.asarray(dec_b, np.float32)
    return out


def _install_ntff_hook():
    """Provide the missing antenv.axon_hooks module so trace=True works
    under axon (replicates trn_boot._ntff_profile_via_ctypes)."""
    import contextlib
    import ctypes
    import types

    if "antenv.axon_hooks" in sys.modules:
        return
    so_path = "/opt/axon/libaxon_pjrt.so"
    holder = {}
    mod = types.ModuleType("antenv.axon_hooks")
    mod.set_axon_ntff_profile_hook = lambda h: holder.__setitem__("h", h)
    mod.get_axon_ntff_profile_hook = lambda: holder.get("h")
    sys.modules["antenv.axon_hooks"] = mod
    try:
        import antenv
        antenv.axon_hooks = mod
    except ImportError:
        pass

    lib = ctypes.CDLL(so_path)
    if not hasattr(lib, "axon_start_nrt_profile"):
        return
    lib.axon_start_nrt_profile.argtypes = [
        ctypes.POINTER(ctypes.c_int64), ctypes.c_size_t]
    lib.axon_start_nrt_profile.restype = ctypes.c_int64
    lib.axon_stop_nrt_profile.argtypes = [ctypes.c_char_p]
    lib.axon_stop_nrt_profile.restype = ctypes.c_int64

    @contextlib.contextmanager
    def _hook(output_dir, device_ids):
        import jax
        jax.devices()
        if device_ids:
            ids = (ctypes.c_int64 * len(device_ids))(*device_ids)
            rc = lib.axon_start_nrt_profile(ids, len(device_ids))
        else:
            rc = lib.axon_start_nrt_profile(None, 0)
        if rc != 0:
            raise RuntimeError(f"axon_start_nrt_profile rc={rc}")
        try:
            yield
        finally:
            n = lib.axon_stop_nrt_profile(str(output_dir).encode())
            print(f"profile: {n} file(s) written to {output_dir}")

    mod.set_axon_ntff_profile_hook(_hook)


def run(x, weight, bias, dec_w, dec_b, trace=False):
    from concourse import bass_utils
    from concourse.bass_utils import run_bass_kernel_spmd

    if trace:
        _install_ntff_hook()
        # artifact upload needs a bucket that doesn't exist here
        bass_utils.upload_artifacts = lambda tmpdir: tmpdir

    nc = _get_nc()
    in_maps = make_in_maps(x, weight, bias, dec_w)
    r = run_bass_kernel_spmd(nc, in_maps, list(range(NCORES)), trace=trace)
    return combine(r.results, dec_b), r


def kernel(x, weight, bias, dec_w, dec_b):
    out, _ = run(x, weight, bias, dec_w, dec_b, trace=False)
    return out



# revision 48
# speedup vs baseline: 1.1411x; 1.0507x over previous
"""Trainium2 Bass kernel for nn_LCN (locally-connected network).

Computation (see module docstring math):
  x: (512, 1, 280, 280) -> non-overlapping 28x28 patches (10x10 grid, P=100)
  y[b, f, p] = sum_{k,l} x[b, 28ph+k, 28pw+l] * w[f*100+p, 0, k, l]
  y = relu(y + bias[f*100+p]);  out = y_flat @ dec_w.T + dec_b   (j = f*100 + p)

Sharding: 8 cores = 4 batch groups x 2 image halves (rows 0..139 | 140..279).
Each core: 128 images, 5 bands (28 rows each), 50 patches.
Per core pipeline:
  - DMA band [128b, 7840] (fp32, contiguous in HBM)
  - PE transpose x chunks [128b, 112pix] -> PSUM [112, 128] (identity matmul)
  - DVE/ACT evacuate PSUM -> SBUF
  - per patch: 7 accumulating matmuls lhsT=w[112,16], rhs=xT[112,128] -> y PSUM
    (4 patches per PSUM tile at partition offsets 0/32/64/96)
  - ACT: relu(y + bias) -> y_sb  [j on partitions, gappy layout]
  - decoder: 13 accumulating matmuls lhsT=dec[128,10] (zeros in gaps) -> out [10,128]
Host sums the two half-image partial decoder outputs and adds dec_b.
"""

import sys

import numpy as np

for _p in ("/opt/trn_rl_repo", "/opt/trn_rl_repo/concourse"):
    if _p not in sys.path:
        sys.path.insert(0, _p)

import concourse.bass as bass
import concourse.mybir as mybir
import concourse.tile as tile
from concourse import bacc
from concourse.masks import make_identity

F32 = mybir.dt.float32

# Problem constants
B, H, W = 512, 280, 280
KS = 28
HS = WS = 10
F = 16
OUT = 10
NCORES = 8
BLOC = 128      # images per core
NBANDS = 5      # bands per core (half image)
NPW = 10        # patches per band
NCHUNK = 7      # 112-pixel chunks per patch (4 rows x 28 cols each)
CK = 112        # contraction chunk size
BAND_W = KS * W  # 7840 elements per band per image


# DMA chunks per band, k-rows each. 14/14 keeps the per-DMA fixed cost
# low (10 x-DMAs; each 15,680 B/partition) — finer chunking measurably
# inflates per-queue DMA busy more than the smoother arrivals save.
CHUNK_K = [14, 14]              # k0-13 | k14-27
CHUNK_KOFF = [0, 14]
# transpose-group phases per band: (t_lo, t_hi, patch groups rel. to band)
TGROUPS = [
    (0, 3, [(0, 1), (2, 3), (4, 5), (6, 7), (8, 9)]),
    (3, 5, [(0, 1, 2, 3), (4, 5, 6, 7), (8, 9)]),
    (5, 7, [(0, 1, 2, 3), (4, 5, 6, 7), (8, 9)]),
]


def build_program(n_bands=NBANDS, n_pw=NPW, use_is_transpose=True, use_bf16=False):
    np_loc = n_bands * n_pw
    ng = (np_loc + 3) // 4
    WDT = mybir.dt.bfloat16 if use_bf16 else F32
    YB = 3                  # y PSUM bufs (max concurrently-open groups)

    nc = bacc.Bacc("TRN2")
    x_d = nc.dram_tensor("x", [BLOC, n_bands * BAND_W], F32, kind="ExternalInput")
    w_d = nc.dram_tensor("w", [CK, np_loc * NCHUNK * F], WDT, kind="ExternalInput")
    b_d = nc.dram_tensor("bias", [128, ng], F32, kind="ExternalInput")
    d_d = nc.dram_tensor("dec", [128, ng * OUT], WDT, kind="ExternalInput")
    o_d = nc.dram_tensor("out", [OUT, BLOC], F32, kind="ExternalOutput")

    with tile.TileContext(nc) as tc:
        with (
            tc.tile_pool(name="const", bufs=1) as constp,
            tc.tile_pool(name="xc0", bufs=3) as xc0p,
            tc.tile_pool(name="xc12", bufs=3) as xc12p,
            tc.tile_pool(name="xpm", bufs=2) as xpmp,
            tc.tile_pool(name="xt", bufs=4) as xtp,
            tc.tile_pool(name="xtps", bufs=4, space="PSUM") as xtpsp,
            tc.tile_pool(name="yps", bufs=YB, space="PSUM") as ypsp,
            tc.tile_pool(name="ops", bufs=1, space="PSUM") as opsp,
        ):
            ident = constp.tile([128, 128], WDT)
            make_identity(nc, ident[:])
            zero_sb = constp.tile([128, 128], F32)
            nc.gpsimd.memset(zero_sb[:], 0.0)
            w_sb = constp.tile([CK, np_loc * NCHUNK * F], WDT)
            bias_sb = constp.tile([128, ng], F32)
            dec_sb = constp.tile([128, ng * OUT], WDT)
            y_sb = constp.tile([128, ng * 128], WDT)
            out_ps = opsp.tile([OUT, BLOC], F32)

            chunk_tiles = {}
            xpm_tiles = {}

            def load_chunk(b, s):
                kc = CHUNK_K[s]
                pool = xc0p if s == 0 else xc12p
                t = pool.tile([128, kc * W], F32, name=f"x_c{min(s, 1)}")
                off = b * BAND_W + CHUNK_KOFF[s] * W
                nc.sync.dma_start(out=t[:], in_=x_d[:, off:off + kc * W])
                chunk_tiles[(b, s)] = t

            def im2col(b, s, pw0, pw1, eng):
                # reorder chunk columns for patches [pw0, pw1) -> patch-
                # major region of x_pm [b, (pw k l)], casting fp32 -> bf16.
                # pw-wise pieces let each PE step gate on one small copy
                # instead of a whole chunk's reorder.
                if b not in xpm_tiles:
                    xpm_tiles[b] = xpmp.tile([128, BAND_W], WDT, name="x_pm")
                t = xpm_tiles[b]
                dst4 = t[:].rearrange("b (pw k l) -> b pw k l", pw=n_pw, k=KS)
                src4 = chunk_tiles[(b, s)][:].rearrange(
                    "b (k pw l) -> b pw k l", k=CHUNK_K[s], pw=n_pw)
                kg = CHUNK_KOFF[s]
                dst = dst4[:, pw0:pw1, kg:kg + CHUNK_K[s], :]
                src = src4[:, pw0:pw1, :, :]
                if eng == "act":
                    nc.scalar.activation(
                        out=dst, in_=src,
                        func=mybir.ActivationFunctionType.Copy)
                else:
                    nc.vector.tensor_copy(dst, src)



            y_tiles = {}

            def emit_mms(group, t_lo, t_hi, xt):
                nt = t_hi - t_lo
                for idx, pp in enumerate(group):
                    G, q = pp // 4, pp % 4
                    if G not in y_tiles:
                        yt = ypsp.tile([128, 128], F32, name="y_ps")
                        if G < YB:
                            # clear stale/NaN PSUM so gaps are finite
                            nc.vector.tensor_copy(yt[:], zero_sb[:])
                        y_tiles[G] = yt
                    yt = y_tiles[G]
                    for t in range(t_lo, t_hi):
                        rhs = xt[:, (idx * nt + t - t_lo) * 128:
                                 (idx * nt + t - t_lo + 1) * 128]
                        nc.tensor.matmul(
                            yt[32 * q:32 * q + F, :],
                            w_sb[:, (pp * NCHUNK + t) * F:
                                 (pp * NCHUNK + t + 1) * F],
                            rhs,
                            start=(t == 0),
                            stop=(t == NCHUNK - 1),
                            tile_position=(0, 32 * q),
                        )
                    if t_hi == NCHUNK and (q == 3 or pp == np_loc - 1):
                        nc.scalar.activation(
                            out=y_sb[:, G * 128:(G + 1) * 128],
                            in_=yt[:],
                            func=mybir.ActivationFunctionType.Relu,
                            bias=bias_sb[:, G:G + 1],
                        )
                        y_tiles.pop(G)
                        # decoder accumulation inline: keeps it off the tail
                        nc.tensor.matmul(
                            out_ps[:],
                            dec_sb[:, G * OUT:(G + 1) * OUT],
                            y_sb[:, G * 128:(G + 1) * 128],
                            start=(G == 0),
                            stop=(G == ng - 1),
                        )

            def transposes(group, t_lo, t_hi, evac_eng):
                # transpose the group's patches for t-range into one PSUM
                # tile, then evacuate with a single DVE copy (amortizes
                # PSUM-access init). Uniform full-bank tiles [CK, 1024].
                band = group[0] // n_pw
                x_pm = xpm_tiles[band]
                nt = t_hi - t_lo
                ps = xtpsp.tile([CK, 1024], WDT, name="xt_ps")
                sb = xtp.tile([CK, 1024], WDT, name="xt")
                for idx, p in enumerate(group):
                    pw = p % n_pw
                    for t in range(t_lo, t_hi):
                        col = idx * nt + t - t_lo
                        dst = ps[:, col * 128:(col + 1) * 128]
                        src = x_pm[:, pw * 784 + t * CK: pw * 784 + (t + 1) * CK]
                        nc.tensor.transpose(dst, src, ident[:])
                ncols = len(group) * nt * 128
                if evac_eng == "act":
                    nc.scalar.activation(
                        out=sb[:, :ncols], in_=ps[:, :ncols],
                        func=mybir.ActivationFunctionType.Copy)
                else:
                    nc.vector.tensor_copy(sb[:, :ncols], ps[:, :ncols])
                return sb

            # depth-2 software pipeline: mains for group i are emitted after
            # the transposes of group i+2, giving the DVE evac two full
            # transpose groups of PE time to complete in
            pending = []

            def step(group, t_lo, t_hi, evac_eng="vec"):
                xt = transposes(group, t_lo, t_hi, evac_eng)
                pending.append((group, t_lo, t_hi, xt))
                if len(pending) > 2:
                    emit_mms(*pending.pop(0))

            WB = n_pw * NCHUNK * F  # w_sb columns per band

            # weights/bias/dec go on the second HWDGE ring (ACT) so they
            # never sit in front of x chunks in the sync-ring FIFO
            def load_w(b):
                nc.sync.dma_start(
                    out=w_sb[:, b * WB:(b + 1) * WB],
                    in_=w_d[:, b * WB:(b + 1) * WB])

            def load_w_rest():
                nc.sync.dma_start(
                    out=w_sb[:, WB:], in_=w_d[:, WB:])

            for band in range(n_bands):
                p0 = band * n_pw
                if band == 0:
                    load_chunk(0, 0)
                    load_w(0)
                    load_chunk(0, 1)
                    nc.sync.dma_start(out=bias_sb[:], in_=b_d[:])
                    if n_bands > 1:
                        load_chunk(1, 0)
                        load_w_rest()
                        load_chunk(1, 1)
                    if n_bands > 2:
                        load_chunk(2, 0)
                        load_chunk(2, 1)
                    nc.sync.dma_start(out=dec_sb[:], in_=d_d[:])
                    # band 0 startup: c0 pieces feed the first A steps
                    im2col(0, 0, 0, 2, "act")
                    im2col(0, 0, 2, 4, "vec")
                    im2col(0, 0, 4, 6, "act")
                # im2col slots: chunks arrive just-in-time; pw-pair pieces
                # gate each PE step on a ~0.7us copy instead of a whole
                # chunk reorder, in slots that don't head-block evacs PE
                # is about to need.
                for gi, (t_lo, t_hi, groups) in enumerate(TGROUPS):
                    for si, g in enumerate(groups):
                        if gi == 0:
                            if si == 0:
                                im2col(band, 0, 6, 8, "vec")
                            elif si == 1:
                                im2col(band, 0, 8, 10, "act")
                            elif si == 2:
                                if band + 3 < n_bands:
                                    load_chunk(band + 3, 0)
                                    load_chunk(band + 3, 1)
                                im2col(band, 1, 0, 2, "act")
                            elif si == 3:
                                im2col(band, 1, 2, 4, "vec")
                            elif si == 4:
                                im2col(band, 1, 4, 6, "act")
                        elif gi == 1:
                            if si == 0:
                                im2col(band, 1, 6, 8, "act")
                            elif si == 1:
                                im2col(band, 1, 8, 10, "act")
                        else:
                            if si == 0 and band + 1 < n_bands:
                                im2col(band + 1, 0, 0, 2, "act")
                            elif si == 1 and band + 1 < n_bands:
                                im2col(band + 1, 0, 2, 4, "vec")
                            elif si == 2 and band + 1 < n_bands:
                                im2col(band + 1, 0, 4, 6, "act")
                        # second quad of each B t-group evacs on ACT to
                        # relieve DVE (its per-band load is near the
                        # stream cadence)
                        eng = "act" if gi > 0 and si == 1 else "vec"
                        step(tuple(p0 + i for i in g), t_lo, t_hi, eng)
            while pending:
                emit_mms(*pending.pop(0))

            out_sb = constp.tile([OUT, BLOC], F32)
            nc.vector.tensor_copy(out_sb[:], out_ps[:])
            nc.sync.dma_start(out=o_d[:], in_=out_sb[:])

    return nc


def stage_half(weight, bias, dec_w, h, n_bands=NBANDS, n_pw=NPW):
    """Host-side staging of weights/bias/decoder for image-half h (0 or 1)."""
    np_loc = n_bands * n_pw
    ng = (np_loc + 3) // 4
    weight = np.asarray(weight, np.float32)
    bias = np.asarray(bias, np.float32)
    dec_w = np.asarray(dec_w, np.float32)

    # w: (1600, 1, 28, 28) -> [f, ph, pw, k, l] -> chunks [d=(kk,l), (bl,pw,t,f)]
    w5 = weight.reshape(F, HS, WS, KS, KS)[:, n_bands * h:n_bands * h + n_bands]
    w6 = w5.reshape(F, n_bands, WS, NCHUNK, 4, KS)  # f bl pw t kk l
    wst = np.ascontiguousarray(
        np.transpose(w6, (4, 5, 1, 2, 3, 0))).reshape(CK, np_loc * NCHUNK * F)

    b5 = bias.reshape(F, HS, WS)[:, n_bands * h:n_bands * h + n_bands, :]
    b5 = b5.reshape(F, np_loc)
    bst = np.zeros((128, ng), np.float32)
    d5 = dec_w.reshape(OUT, F, HS, WS)[:, :, n_bands * h:n_bands * h + n_bands, :]
    d5 = d5.reshape(OUT, F, np_loc)
    dst_ = np.zeros((128, ng * OUT), np.float32)
    for pl in range(np_loc):
        G, q = pl // 4, pl % 4
        bst[32 * q:32 * q + F, G] = b5[:, pl]
        dst_[32 * q:32 * q + F, G * OUT:(G + 1) * OUT] = d5[:, :, pl].T
    return wst, bst, dst_


_cache = {}
USE_BF16 = True
USE_IS_TRANSPOSE = True


def _get_nc():
    key = ("nc", USE_BF16, USE_IS_TRANSPOSE)
    if key not in _cache:
        nc = build_program(use_is_transpose=USE_IS_TRANSPOSE, use_bf16=USE_BF16)
        nc.finalize()
        _cache[key] = nc
    return _cache[key]


def make_in_maps(x, weight, bias, dec_w):
    x = np.asarray(x, np.float32)
    stages = [stage_half(weight, bias, dec_w, h) for h in (0, 1)]
    in_maps = []
    for core in range(NCORES):
        bg, h = core // 2, core % 2
        xs = np.ascontiguousarray(
            x[bg * BLOC:(bg + 1) * BLOC, 0, 140 * h:140 * h + 140, :]
        ).reshape(BLOC, NBANDS * BAND_W)
        wst, bst, dst_ = stages[h]
        if USE_BF16:
            import ml_dtypes
            wst = wst.astype(ml_dtypes.bfloat16)
            dst_ = dst_.astype(ml_dtypes.bfloat16)
        in_maps.append({"x": xs, "w": wst, "bias": bst, "dec": dst_})
    return in_maps


def combine(results, dec_b):
    out = np.zeros((B, OUT), np.float32)
    for bg in range(4):
        part = results[2 * bg]["out"] + results[2 * bg + 1]["out"]  # (10, 128)
        out[bg * BLOC:(bg + 1) * BLOC] = part.T + np.asarray(dec_b, np.float32)
    return out


def _install_ntff_hook():
    """Provide the missing antenv.axon_hooks module so trace=True works
    under axon (replicates trn_boot._ntff_profile_via_ctypes)."""
    import contextlib
    import ctypes
    import types

    if "antenv.axon_hooks" in sys.modules:
        return
    so_path = "/opt/axon/libaxon_pjrt.so"
    holder = {}
    mod = types.ModuleType("antenv.axon_hooks")
    mod.set_axon_ntff_profile_hook = lambda h: holder.__setitem__("h", h)
    mod.get_axon_ntff_profile_hook = lambda: holder.get("h")
    sys.modules["antenv.axon_hooks"] = mod
    try:
        import antenv
        antenv.axon_hooks = mod
    except ImportError:
        pass

    lib = ctypes.CDLL(so_path)
    if not hasattr(lib, "axon_start_nrt_profile"):
        return
    lib.axon_start_nrt_profile.argtypes = [
        ctypes.POINTER(ctypes.c_int64), ctypes.c_size_t]
    lib.axon_start_nrt_profile.restype = ctypes.c_int64
    lib.axon_stop_nrt_profile.argtypes = [ctypes.c_char_p]
    lib.axon_stop_nrt_profile.restype = ctypes.c_int64

    @contextlib.contextmanager
    def _hook(output_dir, device_ids):
        import jax
        jax.devices()
        if device_ids:
            ids = (ctypes.c_int64 * len(device_ids))(*device_ids)
            rc = lib.axon_start_nrt_profile(ids, len(device_ids))
        else:
            rc = lib.axon_start_nrt_profile(None, 0)
        if rc != 0:
            raise RuntimeError(f"axon_start_nrt_profile rc={rc}")
        try:
            yield
        finally:
            n = lib.axon_stop_nrt_profile(str(output_dir).encode())
            print(f"profile: {n} file(s) written to {output_dir}")

    mod.set_axon_ntff_profile_hook(_hook)


def run(x, weight, bias, dec_w, dec_b, trace=False):
    from concourse import bass_utils
    from concourse.bass_utils import run_bass_kernel_spmd

    if trace:
        _install_ntff_hook()
        # artifact upload needs a bucket that doesn't exist here
        bass_utils.upload_artifacts = lambda tmpdir: tmpdir

    nc = _get_nc()
    in_maps = make_in_maps(x, weight, bias, dec_w)
    r = run_bass_kernel_spmd(nc, in_maps, list(range(NCORES)), trace=trace)
    return combine(r.results, dec_b), r


def kernel(x, weight, bias, dec_w, dec_b):
    out, _ = run(x, weight, bias, dec_w, dec_b, trace=False)
    return out

